# revision 23
# baseline (speedup 1.0000x reference)
"""Trainium2 kernel for nn_MultiHeadClassifier.

Math: out[i] = W[task_labels[i]] @ x[i] + b[task_labels[i]]
  x [262144, 1024] f32, task_labels [262144] int, W [8, 32, 1024], b [8, 32]

Strategy (8 NeuronCores, data-parallel over batch):
  - Each core gets 32768 rows. x is staged in HBM transposed
    ([8, 128, 32768]: k-tile, d-within-tile, row) so the PE can contract
    over d (partition dim) directly.
  - All T=8 heads are computed at once per 128-row tile: 8 float32r
    matmuls (full PE rate, ~1.5e-4 rel err) accumulate y = x @ Wflat.T
    ([128 rows, 256]) in PSUM, plus a K=1 bf16 matmul adding the bias.
  - Head selection (the MoE routing) happens on-device on the DVE:
    one-hot mask [128, 8] broadcast-multiplied into y viewed [128, 8, 32],
    then a strided reduce over the 8 task slots -> out tile [128, 32].
  - Output is written in [128, 256, 32] (partition-major) layout with
    fully contiguous per-partition DMA runs; host reshapes back.
"""

import sys

sys.path.insert(0, "/opt/trn_rl_repo")

import numpy as np
import ml_dtypes

import concourse.bass as bass
import concourse.tile as tile
from concourse import bacc, mybir
from concourse import bass_utils

B, D, C, T = 262144, 1024, 32, 8
NCORES = 8
N = B // NCORES  # 32768 rows per core
P = 128
KO = D // P  # 8 contraction tiles
TC = T * C  # 256 = all-heads output width
SB = 512  # rows per superblock (one x DMA)
NT = N // P  # 256 row-tiles per core
SBT = SB // P  # row-tiles per superblock
NSB = N // SB  # superblocks per core

# set by test harness to collect a profile; harness-invoked kernel() keeps it off
TRACE = False
LAST_RESULTS = None


def _build():
    f32 = mybir.dt.float32
    f32r = mybir.dt.float32r
    bf16 = mybir.dt.bfloat16

    nc = bacc.Bacc("TRN2", debug=False, num_devices=NCORES)
    # xt[sb, ki, ko, r]: one superblock is a contiguous 2 MB region with
    # 16 KB contiguous per partition -> near-peak DMA efficiency.
    xt_d = nc.dram_tensor("xt", [NSB, P, KO, SB], f32r, kind="ExternalInput")
    wft_d = nc.dram_tensor("wft", [KO, P, TC], f32r, kind="ExternalInput")
    mask_d = nc.dram_tensor("mask8", [P, NT, T], f32, kind="ExternalInput")
    # bpack[0, :P] = ones, bpack[0, P:] = b.reshape(256) twice (bf16)
    bpack_d = nc.dram_tensor("bpack", [1, P + 2 * TC], bf16, kind="ExternalInput")
    out_d = nc.dram_tensor("out", [P, NT, C], f32, kind="ExternalOutput")

    with tile.TileContext(nc) as tc:
        with (
            tc.tile_pool(name="consts", bufs=1) as consts,
            tc.tile_pool(name="xpool", bufs=6) as xpool,
            tc.tile_pool(name="work", bufs=8) as work,
            tc.tile_pool(name="opool", bufs=3) as opool,
            tc.tile_pool(name="psum", bufs=7, space="PSUM") as psum,
            tc.tile_pool(name="psum_scr", bufs=1, space="PSUM") as psum_scr,
        ):
            # first x superblock in flight before the consts
            xts0 = xpool.tile([P, KO, SB], f32r, tag="xts")
            nc.sync.dma_start(xts0[:], xt_d[0])

            wft = consts.tile([P, KO, TC], f32r)
            nc.sync.dma_start(wft[:], wft_d[:].rearrange("ko ki n -> ki ko n"))
            mask8 = consts.tile([P, NT, T], f32)
            nc.sync.dma_start(mask8[:], mask_d[:])
            bpack = consts.tile([1, P + 2 * TC], bf16)
            nc.sync.dma_start(bpack[:], bpack_d[:])
            ones1 = bpack[:, :P]
            bexp2 = bpack[:, P:]  # [1, 512] = b flat, twice

            # Engine warmups: with the 1-sync-wait-per-instruction ISA
            # limit, give each engine one instruction that observes the
            # const DMA lanes, so steady-state instructions carry at most
            # one wait each.
            scratch = psum_scr.tile([P, 4], mybir.dt.float32, tag="scratch")
            w0 = wft[:, 0, :1].bitcast(bf16)  # [P, 2] garbage bf16 view
            nc.tensor.matmul(scratch[:2, :2], w0, w0, start=True, stop=True)
            dve_scr = work.tile([P, T], f32, tag="dve_scr")
            nc.vector.tensor_copy(dve_scr[:], mask8[:, 0, :])

            for sb in range(NSB):
                if sb == 0:
                    xts = xts0
                else:
                    xts = xpool.tile([P, KO, SB], f32r, tag="xts")
                    nc.sync.dma_start(xts[:], xt_d[sb])
                out_sb = opool.tile([P, SBT, C], f32, tag="out_sb")
                for st in range(SBT):
                    ro = sb * SBT + st
                    y = psum.tile([P, TC], mybir.dt.float32, tag="y")
                    # bias first: absorbs the psum-slot WAR wait; single
                    # const producer (bpack DMA).
                    nc.tensor.matmul(
                        y[:], ones1, bexp2[:, :TC], start=True, stop=False
                    )
                    for ko in range(KO):
                        nc.tensor.matmul(
                            y[:],
                            xts[:, ko, st * P : (st + 1) * P],
                            wft[:, ko, :],
                            start=False,
                            stop=(ko == KO - 1),
                        )
                    # tmp[p, t, c] = y[p, t*C+c] * mask8[p, ro, t]
                    tmp = work.tile([P, TC], f32, tag="tmp")
                    nc.vector.tensor_tensor(
                        tmp[:].rearrange("p (t c) -> p t c", t=T),
                        y[:].rearrange("p (t c) -> p t c", t=T),
                        mask8[:, ro, :, None].to_broadcast((P, T, C)),
                        mybir.AluOpType.mult,
                    )
                    # out[p, c] = sum_t tmp[p, t, c]
                    nc.vector.tensor_reduce(
                        out_sb[:, st, :],
                        tmp[:].rearrange("p (t c) -> p c t", t=T),
                        axis=mybir.AxisListType.X,
                        op=mybir.AluOpType.add,
                    )
                nc.sync.dma_start(
                    out_d[:, sb * SBT : (sb + 1) * SBT, :], out_sb[:]
                )
    nc.compile()
    return nc


_NC = None


def _get_nc():
    global _NC
    if _NC is None:
        _NC = _build()
    return _NC


def kernel(x, task_labels, W, b):
    global LAST_RESULTS
    x = np.asarray(x)
    if x.dtype != np.float32:
        x = x.astype(np.float32)
    labels = np.asarray(task_labels).astype(np.int32)
    W = np.asarray(W)
    if W.dtype != np.float32:
        W = W.astype(np.float32)
    b = np.asarray(b)
    if b.dtype != np.float32:
        b = b.astype(np.float32)

    wft = np.ascontiguousarray(W.reshape(TC, D).T).reshape(KO, P, TC)
    bpack = (
        np.concatenate(
            [np.ones(P, np.float32), b.reshape(TC), b.reshape(TC)]
        )
        .reshape(1, P + 2 * TC)
        .astype(ml_dtypes.bfloat16)
    )
    tids = np.arange(T, dtype=np.int32)[None, None, :]

    in_maps = []
    for c in range(NCORES):
        xs = x[c * N : (c + 1) * N]
        ls = labels[c * N : (c + 1) * N]
        # xt[sb, ki, ko, r] = xs[sb*SB + r, ko*P + ki]
        xt = np.ascontiguousarray(
            xs.reshape(NSB, SB, KO, P).transpose(0, 3, 2, 1)
        )
        lab2 = ls.reshape(NT, P).T  # [P, NT]
        mask8 = (lab2[:, :, None] == tids).astype(np.float32)
        in_maps.append(
            {"xt": xt, "wft": wft, "mask8": mask8, "bpack": bpack}
        )

    nc = _get_nc()
    res = bass_utils.run_bass_kernel_spmd(
        nc, in_maps, core_ids=list(range(NCORES)), trace=TRACE
    )
    LAST_RESULTS = res
    outs = [
        r["out"].transpose(1, 0, 2).reshape(N, C) for r in res.results
    ]
    return np.concatenate(outs, axis=0)


# revision 25
# speedup vs baseline: 1.2878x; 1.2878x over previous
"""Trainium2 kernel for nn_MultiHeadClassifier.

Math: out[i] = W[task_labels[i]] @ x[i] + b[task_labels[i]]
  x [262144, 1024] f32, task_labels [262144] int, W [8, 32, 1024], b [8, 32]

Strategy (8 NeuronCores, data-parallel over batch):
  - Each core gets 32768 rows. x is staged in HBM transposed
    ([8, 128, 32768]: k-tile, d-within-tile, row) so the PE can contract
    over d (partition dim) directly.
  - All T=8 heads are computed at once per 128-row tile: 8 float32r
    matmuls (full PE rate, ~1.5e-4 rel err) accumulate y = x @ Wflat.T
    ([128 rows, 256]) in PSUM, plus a K=1 bf16 matmul adding the bias.
  - Head selection (the MoE routing) happens on-device on the DVE:
    one-hot mask [128, 8] broadcast-multiplied into y viewed [128, 8, 32],
    then a strided reduce over the 8 task slots -> out tile [128, 32].
  - Output is written in [128, 256, 32] (partition-major) layout with
    fully contiguous per-partition DMA runs; host reshapes back.
"""

import sys

sys.path.insert(0, "/opt/trn_rl_repo")

import numpy as np
import ml_dtypes

import concourse.bass as bass
import concourse.tile as tile
from concourse import bacc, mybir
from concourse import bass_utils

B, D, C, T = 262144, 1024, 32, 8
NCORES = 8
N = B // NCORES  # 32768 rows per core
P = 128
KO = D // P  # 8 contraction tiles
TC = T * C  # 256 = all-heads output width
SB = 1024  # rows per superblock (one x DMA)
NT = N // P  # 256 row-tiles per core
SBT = SB // P  # row-tiles per superblock
NSB = N // SB  # superblocks per core

# set by test harness to collect a profile; harness-invoked kernel() keeps it off
TRACE = False
LAST_RESULTS = None


def _build():
    f32 = mybir.dt.float32
    f32r = mybir.dt.float32r
    bf16 = mybir.dt.bfloat16

    nc = bacc.Bacc("TRN2", debug=False, num_devices=NCORES)
    # xt[sb, ki, ko, r]: one superblock is a contiguous 2 MB region with
    # 16 KB contiguous per partition -> near-peak DMA efficiency.
    xt_d = nc.dram_tensor("xt", [NSB, P, KO, SB], f32r, kind="ExternalInput")
    wft_d = nc.dram_tensor("wft", [KO, P, TC], f32r, kind="ExternalInput")
    mask_d = nc.dram_tensor("mask8", [P, NT, T], f32, kind="ExternalInput")
    # bpack[0, :P] = ones, bpack[0, P:] = b.reshape(256) twice (bf16)
    bpack_d = nc.dram_tensor("bpack", [1, P + 2 * TC], bf16, kind="ExternalInput")
    out_d = nc.dram_tensor("out", [P, NT, C], f32, kind="ExternalOutput")

    with tile.TileContext(nc) as tc:
        with (
            tc.tile_pool(name="consts", bufs=1) as consts,
            tc.tile_pool(name="xpool", bufs=4) as xpool,
            tc.tile_pool(name="work", bufs=8) as work,
            tc.tile_pool(name="opool", bufs=3) as opool,
            tc.tile_pool(name="psum", bufs=7, space="PSUM") as psum,
            tc.tile_pool(name="psum_scr", bufs=1, space="PSUM") as psum_scr,
        ):
            # first x superblock in flight before the consts
            xts0 = xpool.tile([P, KO, SB], f32r, tag="xts")
            nc.sync.dma_start(xts0[:], xt_d[0])

            wft = consts.tile([P, KO, TC], f32r)
            nc.sync.dma_start(wft[:], wft_d[:].rearrange("ko ki n -> ki ko n"))
            mask8 = consts.tile([P, NT, T], f32)
            nc.sync.dma_start(mask8[:], mask_d[:])
            bpack = consts.tile([1, P + 2 * TC], bf16)
            nc.sync.dma_start(bpack[:], bpack_d[:])
            ones1 = bpack[:, :P]
            bexp2 = bpack[:, P:]  # [1, 512] = b flat, twice

            # Engine warmups: with the 1-sync-wait-per-instruction ISA
            # limit, give each engine one instruction that observes the
            # const DMA lanes, so steady-state instructions carry at most
            # one wait each.
            scratch = psum_scr.tile([P, 4], mybir.dt.float32, tag="scratch")
            w0 = wft[:, 0, :1].bitcast(bf16)  # [P, 2] garbage bf16 view
            nc.tensor.matmul(scratch[:2, :2], w0, w0, start=True, stop=True)
            dve_scr = work.tile([P, T], f32, tag="dve_scr")
            nc.vector.tensor_copy(dve_scr[:], mask8[:, 0, :])

            for sb in range(NSB):
                if sb == 0:
                    xts = xts0
                else:
                    xts = xpool.tile([P, KO, SB], f32r, tag="xts")
                    nc.sync.dma_start(xts[:], xt_d[sb])
                out_sb = opool.tile([P, SBT, C], f32, tag="out_sb")
                for st in range(SBT):
                    ro = sb * SBT + st
                    y = psum.tile([P, TC], mybir.dt.float32, tag="y")
                    # bias first: absorbs the psum-slot WAR wait; single
                    # const producer (bpack DMA).
                    nc.tensor.matmul(
                        y[:], ones1, bexp2[:, :TC], start=True, stop=False
                    )
                    for ko in range(KO):
                        nc.tensor.matmul(
                            y[:],
                            xts[:, ko, st * P : (st + 1) * P],
                            wft[:, ko, :],
                            start=False,
                            stop=(ko == KO - 1),
                        )
                    # tmp[p, t, c] = y[p, t*C+c] * mask8[p, ro, t]
                    tmp = work.tile([P, TC], f32, tag="tmp")
                    nc.vector.tensor_tensor(
                        tmp[:].rearrange("p (t c) -> p t c", t=T),
                        y[:].rearrange("p (t c) -> p t c", t=T),
                        mask8[:, ro, :, None].to_broadcast((P, T, C)),
                        mybir.AluOpType.mult,
                    )
                    # out[p, c] = sum_t tmp[p, t, c]
                    nc.vector.tensor_reduce(
                        out_sb[:, st, :],
                        tmp[:].rearrange("p (t c) -> p c t", t=T),
                        axis=mybir.AxisListType.X,
                        op=mybir.AluOpType.add,
                    )
                nc.sync.dma_start(
                    out_d[:, sb * SBT : (sb + 1) * SBT, :], out_sb[:]
                )
    nc.compile()
    return nc


_NC = None


def _get_nc():
    global _NC
    if _NC is None:
        _NC = _build()
    return _NC


def kernel(x, task_labels, W, b):
    global LAST_RESULTS
    x = np.asarray(x)
    if x.dtype != np.float32:
        x = x.astype(np.float32)
    labels = np.asarray(task_labels).astype(np.int32)
    W = np.asarray(W)
    if W.dtype != np.float32:
        W = W.astype(np.float32)
    b = np.asarray(b)
    if b.dtype != np.float32:
        b = b.astype(np.float32)

    wft = np.ascontiguousarray(W.reshape(TC, D).T).reshape(KO, P, TC)
    bpack = (
        np.concatenate(
            [np.ones(P, np.float32), b.reshape(TC), b.reshape(TC)]
        )
        .reshape(1, P + 2 * TC)
        .astype(ml_dtypes.bfloat16)
    )
    tids = np.arange(T, dtype=np.int32)[None, None, :]

    in_maps = []
    for c in range(NCORES):
        xs = x[c * N : (c + 1) * N]
        ls = labels[c * N : (c + 1) * N]
        # xt[sb, ki, ko, r] = xs[sb*SB + r, ko*P + ki]
        xt = np.ascontiguousarray(
            xs.reshape(NSB, SB, KO, P).transpose(0, 3, 2, 1)
        )
        lab2 = ls.reshape(NT, P).T  # [P, NT]
        mask8 = (lab2[:, :, None] == tids).astype(np.float32)
        in_maps.append(
            {"xt": xt, "wft": wft, "mask8": mask8, "bpack": bpack}
        )

    nc = _get_nc()
    res = bass_utils.run_bass_kernel_spmd(
        nc, in_maps, core_ids=list(range(NCORES)), trace=TRACE
    )
    LAST_RESULTS = res
    outs = [
        r["out"].transpose(1, 0, 2).reshape(N, C) for r in res.results
    ]
    return np.concatenate(outs, axis=0)


# revision 27
# speedup vs baseline: 1.3125x; 1.0191x over previous
"""Trainium2 kernel for nn_MultiHeadClassifier.

Math: out[i] = W[task_labels[i]] @ x[i] + b[task_labels[i]]
  x [262144, 1024] f32, task_labels [262144] int, W [8, 32, 1024], b [8, 32]

Strategy (8 NeuronCores, data-parallel over batch):
  - Each core gets 32768 rows. x is staged in HBM transposed
    ([8, 128, 32768]: k-tile, d-within-tile, row) so the PE can contract
    over d (partition dim) directly.
  - All T=8 heads are computed at once per 128-row tile: 8 float32r
    matmuls (full PE rate, ~1.5e-4 rel err) accumulate y = x @ Wflat.T
    ([128 rows, 256]) in PSUM, plus a K=1 bf16 matmul adding the bias.
  - Head selection (the MoE routing) happens on-device on the DVE:
    one-hot mask [128, 8] broadcast-multiplied into y viewed [128, 8, 32],
    then a strided reduce over the 8 task slots -> out tile [128, 32].
  - Output is written in [128, 256, 32] (partition-major) layout with
    fully contiguous per-partition DMA runs; host reshapes back.
"""

import sys

sys.path.insert(0, "/opt/trn_rl_repo")

import numpy as np
import ml_dtypes

import concourse.bass as bass
import concourse.tile as tile
from concourse import bacc, mybir
from concourse import bass_utils

B, D, C, T = 262144, 1024, 32, 8
NCORES = 8
N = B // NCORES  # 32768 rows per core
P = 128
KO = D // P  # 8 contraction tiles
TC = T * C  # 256 = all-heads output width
SB = 1024  # rows per superblock (one x DMA)
NT = N // P  # 256 row-tiles per core
SBT = SB // P  # row-tiles per superblock
NSB = N // SB  # superblocks per core

# set by test harness to collect a profile; harness-invoked kernel() keeps it off
TRACE = False
LAST_RESULTS = None


def _build():
    f32 = mybir.dt.float32
    f32r = mybir.dt.float32r
    bf16 = mybir.dt.bfloat16

    nc = bacc.Bacc("TRN2", debug=False, num_devices=NCORES)
    # xt[sb, ki, ko, r]: one superblock is a contiguous 2 MB region with
    # 16 KB contiguous per partition -> near-peak DMA efficiency.
    xt_d = nc.dram_tensor("xt", [NSB, P, KO, SB], f32r, kind="ExternalInput")
    wft_d = nc.dram_tensor("wft", [KO, P, TC], f32r, kind="ExternalInput")
    mask_d = nc.dram_tensor("mask8", [P, NT, T], f32, kind="ExternalInput")
    # bpack[0, :P] = ones, bpack[0, P:] = b.reshape(256) twice (bf16)
    bpack_d = nc.dram_tensor("bpack", [1, P + 2 * TC], bf16, kind="ExternalInput")
    out_d = nc.dram_tensor("out", [P, NT, C], f32, kind="ExternalOutput")

    with tile.TileContext(nc) as tc:
        with (
            tc.tile_pool(name="consts", bufs=1) as consts,
            tc.tile_pool(name="xpool", bufs=5) as xpool,
            tc.tile_pool(name="work", bufs=8) as work,
            tc.tile_pool(name="opool", bufs=3) as opool,
            tc.tile_pool(name="psum", bufs=7, space="PSUM") as psum,
            tc.tile_pool(name="psum_scr", bufs=1, space="PSUM") as psum_scr,
        ):
            # first x superblock in flight before the consts
            xts0 = xpool.tile([P, KO, SB], f32r, tag="xts")
            nc.sync.dma_start(xts0[:], xt_d[0])

            wft = consts.tile([P, KO, TC], f32r)
            nc.sync.dma_start(wft[:], wft_d[:].rearrange("ko ki n -> ki ko n"))
            mask8 = consts.tile([P, NT, T], f32)
            nc.sync.dma_start(mask8[:], mask_d[:])
            bpack = consts.tile([1, P + 2 * TC], bf16)
            nc.sync.dma_start(bpack[:], bpack_d[:])
            ones1 = bpack[:, :P]
            bexp2 = bpack[:, P:]  # [1, 512] = b flat, twice

            # Engine warmups: with the 1-sync-wait-per-instruction ISA
            # limit, give each engine one instruction that observes the
            # const DMA lanes, so steady-state instructions carry at most
            # one wait each.
            scratch = psum_scr.tile([P, 4], mybir.dt.float32, tag="scratch")
            w0 = wft[:, 0, :1].bitcast(bf16)  # [P, 2] garbage bf16 view
            nc.tensor.matmul(scratch[:2, :2], w0, w0, start=True, stop=True)
            dve_scr = work.tile([P, T], f32, tag="dve_scr")
            nc.vector.tensor_copy(dve_scr[:], mask8[:, 0, :])

            for sb in range(NSB):
                if sb == 0:
                    xts = xts0
                else:
                    xts = xpool.tile([P, KO, SB], f32r, tag="xts")
                    nc.sync.dma_start(xts[:], xt_d[sb])
                out_sb = opool.tile([P, SBT, C], f32, tag="out_sb")
                for st in range(SBT):
                    ro = sb * SBT + st
                    y = psum.tile([P, TC], mybir.dt.float32, tag="y")
                    # bias first: absorbs the psum-slot WAR wait; single
                    # const producer (bpack DMA).
                    nc.tensor.matmul(
                        y[:], ones1, bexp2[:, :TC], start=True, stop=False
                    )
                    for ko in range(KO):
                        nc.tensor.matmul(
                            y[:],
                            xts[:, ko, st * P : (st + 1) * P],
                            wft[:, ko, :],
                            start=False,
                            stop=(ko == KO - 1),
                        )
                    # tmp[p, t, c] = y[p, t*C+c] * mask8[p, ro, t]
                    tmp = work.tile([P, TC], f32, tag="tmp")
                    nc.vector.tensor_tensor(
                        tmp[:].rearrange("p (t c) -> p t c", t=T),
                        y[:].rearrange("p (t c) -> p t c", t=T),
                        mask8[:, ro, :, None].to_broadcast((P, T, C)),
                        mybir.AluOpType.mult,
                    )
                    # out[p, c] = sum_t tmp[p, t, c]
                    nc.vector.tensor_reduce(
                        out_sb[:, st, :],
                        tmp[:].rearrange("p (t c) -> p c t", t=T),
                        axis=mybir.AxisListType.X,
                        op=mybir.AluOpType.add,
                    )
                # out on the ACT HWDGE ring so it never delays xts loads
                # queued on the SP ring
                nc.scalar.dma_start(
                    out_d[:, sb * SBT : (sb + 1) * SBT, :], out_sb[:]
                )
    nc.compile()
    return nc


_NC = None


def _get_nc():
    global _NC
    if _NC is None:
        _NC = _build()
    return _NC


def kernel(x, task_labels, W, b):
    global LAST_RESULTS
    x = np.asarray(x)
    if x.dtype != np.float32:
        x = x.astype(np.float32)
    labels = np.asarray(task_labels).astype(np.int32)
    W = np.asarray(W)
    if W.dtype != np.float32:
        W = W.astype(np.float32)
    b = np.asarray(b)
    if b.dtype != np.float32:
        b = b.astype(np.float32)

    wft = np.ascontiguousarray(W.reshape(TC, D).T).reshape(KO, P, TC)
    bpack = (
        np.concatenate(
            [np.ones(P, np.float32), b.reshape(TC), b.reshape(TC)]
        )
        .reshape(1, P + 2 * TC)
        .astype(ml_dtypes.bfloat16)
    )
    tids = np.arange(T, dtype=np.int32)[None, None, :]

    in_maps = []
    for c in range(NCORES):
        xs = x[c * N : (c + 1) * N]
        ls = labels[c * N : (c + 1) * N]
        # xt[sb, ki, ko, r] = xs[sb*SB + r, ko*P + ki]
        xt = np.ascontiguousarray(
            xs.reshape(NSB, SB, KO, P).transpose(0, 3, 2, 1)
        )
        lab2 = ls.reshape(NT, P).T  # [P, NT]
        mask8 = (lab2[:, :, None] == tids).astype(np.float32)
        in_maps.append(
            {"xt": xt, "wft": wft, "mask8": mask8, "bpack": bpack}
        )

    nc = _get_nc()
    res = bass_utils.run_bass_kernel_spmd(
        nc, in_maps, core_ids=list(range(NCORES)), trace=TRACE
    )
    LAST_RESULTS = res
    outs = [
        r["out"].transpose(1, 0, 2).reshape(N, C) for r in res.results
    ]
    return np.concatenate(outs, axis=0)


# revision 30
# speedup vs baseline: 1.3954x; 1.0632x over previous
"""Trainium2 kernel for nn_MultiHeadClassifier.

Math: out[i] = W[task_labels[i]] @ x[i] + b[task_labels[i]]
  x [262144, 1024] f32, task_labels [262144] int, W [8, 32, 1024], b [8, 32]

Strategy (8 NeuronCores, data-parallel over batch):
  - Each core gets 32768 rows. x is staged in HBM transposed
    ([8, 128, 32768]: k-tile, d-within-tile, row) so the PE can contract
    over d (partition dim) directly.
  - All T=8 heads are computed at once per 128-row tile: 8 float32r
    matmuls (full PE rate, ~1.5e-4 rel err) accumulate y = x @ Wflat.T
    ([128 rows, 256]) in PSUM, plus a K=1 bf16 matmul adding the bias.
  - Head selection (the MoE routing) happens on-device on the DVE:
    one-hot mask [128, 8] broadcast-multiplied into y viewed [128, 8, 32],
    then a strided reduce over the 8 task slots -> out tile [128, 32].
  - Output is written in [128, 256, 32] (partition-major) layout with
    fully contiguous per-partition DMA runs; host reshapes back.
"""

import sys

sys.path.insert(0, "/opt/trn_rl_repo")

import numpy as np
import ml_dtypes

import concourse.bass as bass
import concourse.tile as tile
from concourse import bacc, mybir
from concourse import bass_utils

B, D, C, T = 262144, 1024, 32, 8
NCORES = 8
N = B // NCORES  # 32768 rows per core
P = 128
KO = D // P  # 8 contraction tiles
TC = T * C  # 256 = all-heads output width
SB = 1024  # rows per superblock (one x DMA)
NT = N // P  # 256 row-tiles per core
SBT = SB // P  # row-tiles per superblock
NSB = N // SB  # superblocks per core

# set by test harness to collect a profile; harness-invoked kernel() keeps it off
TRACE = False
LAST_RESULTS = None


def _build():
    f32 = mybir.dt.float32
    f32r = mybir.dt.float32r
    bf16 = mybir.dt.bfloat16

    nc = bacc.Bacc("TRN2", debug=False, num_devices=NCORES)
    # xt[sb, ki, ko, r]: one superblock is a contiguous 2 MB region with
    # 16 KB contiguous per partition -> near-peak DMA efficiency.
    xt_d = nc.dram_tensor("xt", [NSB, P, KO, SB], f32r, kind="ExternalInput")
    wft_d = nc.dram_tensor("wft", [KO, P, TC], f32r, kind="ExternalInput")
    mask_d = nc.dram_tensor("mask8", [P, NT, T], f32, kind="ExternalInput")
    # bpack[0, :P] = ones, bpack[0, P:] = b.reshape(256) twice (bf16)
    bpack_d = nc.dram_tensor("bpack", [1, P + 2 * TC], bf16, kind="ExternalInput")
    out_d = nc.dram_tensor("out", [P, NT, C], f32, kind="ExternalOutput")

    with tile.TileContext(nc) as tc:
        with (
            tc.tile_pool(name="consts", bufs=1) as consts,
            tc.tile_pool(name="xpool", bufs=5) as xpool,
            tc.tile_pool(name="work", bufs=8) as work,
            tc.tile_pool(name="opool", bufs=3) as opool,
            tc.tile_pool(name="psum", bufs=8, space="PSUM") as psum,
        ):
            # first x superblock in flight before the consts
            xts0 = xpool.tile([P, KO, SB], f32r, tag="xts")
            nc.sync.dma_start(xts0[:], xt_d[0])

            # consts on the ACT ring: the SP ring stays a pure x stream
            wft = consts.tile([P, KO, TC], f32r)
            nc.scalar.dma_start(wft[:], wft_d[:].rearrange("ko ki n -> ki ko n"))
            mask8 = consts.tile([P, NT, T], f32)
            nc.scalar.dma_start(mask8[:], mask_d[:])
            bpack = consts.tile([1, P + 2 * TC], bf16)
            nc.scalar.dma_start(bpack[:], bpack_d[:])
            ones1 = bpack[:, :P]
            bexp2 = bpack[:, P:]  # [1, 512] = b flat, twice

            # Engine warmups: with the 1-sync-wait-per-instruction ISA
            # limit, give each engine one instruction that observes the
            # const DMA lanes, so steady-state instructions carry at most
            # one wait each.
            scratch = psum.tile([P, TC], mybir.dt.float32, tag="y")
            w0 = wft[:, 0, :1].bitcast(bf16)  # [P, 2] garbage bf16 view
            nc.tensor.matmul(scratch[:2, :2], w0, w0, start=True, stop=True)
            dve_scr = work.tile([P, T], f32, tag="dve_scr")
            nc.vector.tensor_copy(dve_scr[:], mask8[:, 0, :])

            for sb in range(NSB):
                if sb == 0:
                    xts = xts0
                else:
                    xts = xpool.tile([P, KO, SB], f32r, tag="xts")
                    nc.sync.dma_start(xts[:], xt_d[sb])
                out_sb = opool.tile([P, SBT, C], f32, tag="out_sb")
                for st in range(SBT):
                    ro = sb * SBT + st
                    y = psum.tile([P, TC], mybir.dt.float32, tag="y")
                    # bias first: absorbs the psum-slot WAR wait; single
                    # const producer (bpack DMA).
                    nc.tensor.matmul(
                        y[:], ones1, bexp2[:, :TC], start=True, stop=False
                    )
                    for ko in range(KO):
                        nc.tensor.matmul(
                            y[:],
                            xts[:, ko, st * P : (st + 1) * P],
                            wft[:, ko, :],
                            start=False,
                            stop=(ko == KO - 1),
                        )
                    # tmp[p, t, c] = y[p, t*C+c] * mask8[p, ro, t]
                    tmp = work.tile([P, TC], f32, tag="tmp")
                    nc.vector.tensor_tensor(
                        tmp[:].rearrange("p (t c) -> p t c", t=T),
                        y[:].rearrange("p (t c) -> p t c", t=T),
                        mask8[:, ro, :, None].to_broadcast((P, T, C)),
                        mybir.AluOpType.mult,
                    )
                    # out[p, c] = sum_t tmp[p, t, c]
                    nc.vector.tensor_reduce(
                        out_sb[:, st, :],
                        tmp[:].rearrange("p (t c) -> p c t", t=T),
                        axis=mybir.AxisListType.X,
                        op=mybir.AluOpType.add,
                    )
                # out on the ACT HWDGE ring so it never delays xts loads
                # queued on the SP ring
                nc.scalar.dma_start(
                    out_d[:, sb * SBT : (sb + 1) * SBT, :], out_sb[:]
                )
    nc.compile()
    return nc


_NC = None


def _get_nc():
    global _NC
    if _NC is None:
        _NC = _build()
    return _NC


def kernel(x, task_labels, W, b):
    global LAST_RESULTS
    x = np.asarray(x)
    if x.dtype != np.float32:
        x = x.astype(np.float32)
    labels = np.asarray(task_labels).astype(np.int32)
    W = np.asarray(W)
    if W.dtype != np.float32:
        W = W.astype(np.float32)
    b = np.asarray(b)
    if b.dtype != np.float32:
        b = b.astype(np.float32)

    wft = np.ascontiguousarray(W.reshape(TC, D).T).reshape(KO, P, TC)
    bpack = (
        np.concatenate(
            [np.ones(P, np.float32), b.reshape(TC), b.reshape(TC)]
        )
        .reshape(1, P + 2 * TC)
        .astype(ml_dtypes.bfloat16)
    )
    tids = np.arange(T, dtype=np.int32)[None, None, :]

    in_maps = []
    for c in range(NCORES):
        xs = x[c * N : (c + 1) * N]
        ls = labels[c * N : (c + 1) * N]
        # xt[sb, ki, ko, r] = xs[sb*SB + r, ko*P + ki]
        xt = np.ascontiguousarray(
            xs.reshape(NSB, SB, KO, P).transpose(0, 3, 2, 1)
        )
        lab2 = ls.reshape(NT, P).T  # [P, NT]
        mask8 = (lab2[:, :, None] == tids).astype(np.float32)
        in_maps.append(
            {"xt": xt, "wft": wft, "mask8": mask8, "bpack": bpack}
        )

    nc = _get_nc()
    res = bass_utils.run_bass_kernel_spmd(
        nc, in_maps, core_ids=list(range(NCORES)), trace=TRACE
    )
    LAST_RESULTS = res
    outs = [
        r["out"].transpose(1, 0, 2).reshape(N, C) for r in res.results
    ]
    return np.concatenate(outs, axis=0)
